# revision 27
# baseline (speedup 1.0000x reference)
"""Trainium2 Bass kernel for 3D multi-head attention (nn_Attention3D).

Problem: x [1, 16, 16, 16, 528] -> full attention over N=4096 tokens,
8 heads of dim 66, qkv + out projections.

Sharding: one head per NeuronCore (8 cores). Each core computes its
head's q/k/v projections, full 4096x4096 attention, and its partial
contribution to the output projection. Host divides each core's
partial by its softmax denominator (carried as an extra output
column), sums the 8 partials and adds the output bias.

Final pipeline (fp8 AV + dual-engine exp; ~171us HW, vs 201us v1):
  - scores and projections stay bf16: on this silicon fp8 DoubleRow
    streams one moving column per cycle (like bf16) and disables the
    fast-weight-load, so fp8 matmuls only pay off where they merge two
    instructions into one. AV does exactly that: each AV is one
    DoubleRow matmul over two 128-token k-chunks (vaug [128,2k,80]
    stationary, E [128,2,512] moving), halving AV instruction count.
  - exp of the 16.7M scores is split across two engines (the ~109us
    ScalarE-only exp was the v1 co-bottleneck): ScalarE runs native
    Exp into fp8 E tiles (~10/16 pairs per block), DVE runs a one-op
    Schraudolph on the rest: int8(A*s + B) written through a bitcast
    into the fp8 tile -- the int8 grid IS the fp8e4m3
    exponent/mantissa grid, so the linear-in-log approximation lands
    within fp8 rounding error. DVE also carries the y/oT/kT/v casts,
    ScalarE the qT casts.
  - scores are emitted in uniform 2-chunk pairs ([128,2,512] f32 PSUM
    = 2 banks, ring of 3, shared with the out-proj pieces); o_ps
    [80,512] ring-2; the deferred-work FIFO (AV, oT cast, out-proj
    pieces) replays a block behind the score/exp pipeline.
  - v is quantized to fp8 at the PSUM->SBUF copy into an 80-stride
    layout (DoubleRow k-tile step must be %16==0), with a ones column
    so the softmax denominator rides the AV accumulator row 0.
  - x loads in a few wide strided DMAs (per-dma_start issue costs
    ~650ns on SyncE); 90 warmup matmuls hold the PE p-state until
    block 0's x lands; y leaves as one contiguous DMA per 512-token
    block in a [block][partition][piece][col] dram layout the host
    untangles; the last block drains piecewise and splits its final
    exps across both engines to shorten the tail.
Phase B runs at the PE streaming roofline (~14.4us per 512-token
block: 16384 score cols + 16384 AV cols at ~2.4GHz, 1 col/cycle).
Measured rel err vs fp32 reference ~8.3e-3.
"""

import numpy as np

import ml_dtypes

BF16_NP = ml_dtypes.bfloat16
FP8_NP = ml_dtypes.float8_e4m3

EMBED = 528
EOUT = 536  # proj output cols: 528 data + denom col (528) + pad
HD = 66
NHEADS = 8
NT = 4096
NCH = 5  # contraction chunks of 128 (640 = 528 + bias row + pad)

# Schraudolph fast-exp constants: scores arrive pre-scaled by hd^-0.5
# (folded into wq), so A is just 8*log2(e) onto the int8/fp8e4m3 grid.
A_SCH = 8.0 * np.log2(np.e)
B_SCH = 56.0 - 0.35

# exp-engine assignment per score pair (16 pairs of k-chunks per block):
# pairs listed go to DVE (Schraudolph), the rest to ScalarE Exp. DVE
# also carries the y/oT/kT/v casts, so it gets the smaller share.
DVE_PAIRS_B0 = frozenset({1, 3, 5, 7, 9, 11, 13})  # 7/16 in block 0
DVE_PAIRS = frozenset({1, 4, 7, 10, 12, 15})  # 6/16 in blocks 1-7


def _build_nc(nt=NT):
    import concourse.tile as tile
    from concourse import bacc, mybir

    F32 = mybir.dt.float32
    BF16 = mybir.dt.bfloat16
    FP8 = mybir.dt.float8e4
    I8 = mybir.dt.int8
    AF = mybir.ActivationFunctionType
    DR = mybir.MatmulPerfMode.DoubleRow
    MULT = mybir.AluOpType.mult
    ADD = mybir.AluOpType.add

    nkc = nt // 128  # k-token chunks (32)
    npair = nkc // 2  # AV pairs per block (16)
    nqb = nt // 512  # q-token blocks (8)

    nc = bacc.Bacc("TRN2", target_bir_lowering=False, debug=False)
    xT_d = nc.dram_tensor("xT", [128, NCH, nt], BF16, kind="ExternalInput").ap()
    wq_d = nc.dram_tensor("wq", [128, NCH, 128], BF16, kind="ExternalInput").ap()
    wk_d = nc.dram_tensor("wk", [128, NCH, 128], BF16, kind="ExternalInput").ap()
    wv_d = nc.dram_tensor("wv", [128, NCH, HD + 2], BF16, kind="ExternalInput").ap()
    wp_d = nc.dram_tensor("wp", [128, EOUT], BF16, kind="ExternalInput").ap()
    y_d = nc.dram_tensor(
        "y", [nt // 512, 128, 4, EOUT], BF16, kind="ExternalOutput"
    ).ap()

    with tile.TileContext(nc) as tc:
        with (
            tc.tile_pool(name="const", bufs=1) as constp,
            tc.tile_pool(name="persist", bufs=1) as pp,
            tc.tile_pool(name="ep", bufs=8) as ep,
            tc.tile_pool(name="yp", bufs=4) as yp,
            tc.tile_pool(name="psS", bufs=1, space="PSUM") as psS,
        ):
            wq = constp.tile([128, NCH, 128], BF16, name="wq_sb")
            wk = constp.tile([128, NCH, 128], BF16, name="wk_sb")
            wv = constp.tile([128, NCH, HD + 2], BF16, name="wv_sb")
            wp = constp.tile([128, EOUT], BF16, name="wp_sb")
            warm = constp.tile([128, 16], BF16, name="warm_sb")

            xT = pp.tile([128, NCH, nt], BF16, name="xT_sb")
            # block 0's x + qk weights first so qk(0) starts ~9us in;
            # the rest of x in wide DMAs that land on other queues
            nc.sync.dma_start(wq[:], wq_d[:])
            nc.sync.dma_start(xT[:, 0:2, 0:512], xT_d[:, 0:2, 0:512])
            nc.sync.dma_start(xT[:, 2:NCH, 0:512], xT_d[:, 2:NCH, 0:512])
            nc.sync.dma_start(wk[:], wk_d[:])
            nc.sync.dma_start(wv[:], wv_d[:])
            nc.sync.dma_start(xT[:, :, 512:2048], xT_d[:, :, 512:2048])
            nc.sync.dma_start(xT[:, :, 2048:nt], xT_d[:, :, 2048:nt])
            nc.sync.dma_start(wp[:], wp_d[:])

            # qT/kT are hd-padded to 128 partitions (rows HD.. stay 0) so
            # scores contract over a full K=128.
            qT = pp.tile([128, nt], BF16, name="qT")
            kT = pp.tile([128, nt], BF16, name="kT")
            # v in fp8 with an 80-byte chunk stride (DoubleRow k-tile dim
            # step must be a multiple of 16); cols 68-79 stay zero.
            vaug = pp.tile([128, nkc, 80], FP8, name="vaug")
            # out-proj stationary per block, double-buffered; rows 68-127
            # must read zero in the proj matmul, so memset once and only
            # ever write rows 0..67.
            oT = [pp.tile([128, 512], BF16, name=f"oT{i}") for i in range(2)]
            nc.gpsimd.memset(warm[:], 0)
            nc.gpsimd.memset(vaug[:], 0)
            nc.gpsimd.memset(oT[0][:], 0)
            nc.gpsimd.memset(oT[1][:], 0)

            # ---- deferred-work FIFO: AV pairs, oT casts, projections ----
            o_ps_tiles = {}
            ysb_tiles = {}
            avq = []

            def pop_work(budget, floor=0):
                spent = 0
                while len(avq) > floor and spent < budget:
                    item = avq[0]
                    kind = item[0]
                    if kind == "av":
                        _, b, E, j = item
                        if b not in o_ps_tiles:
                            o_ps_tiles[b] = psO.tile(
                                [80, 512], F32, tag="o", bufs=2, name="o_ps"
                            )
                        o_ps = o_ps_tiles[b]
                        nc.tensor.matmul(
                            o_ps[:],
                            vaug[:, 2 * j : 2 * j + 2, :],
                            E[:],
                            start=(j == 0),
                            stop=(j == npair - 1),
                            perf_mode=DR,
                            skip_group_check=True,
                        )
                        spent += 1
                    elif kind == "cast":
                        b = item[1]
                        nc.vector.tensor_copy(
                            oT[b % 2][0 : HD + 2, :], o_ps_tiles[b][0 : HD + 2, :]
                        )
                    else:  # proj piece: one 128-token slice of block b
                        _, b, t = item
                        oTt = oT[b % 2]
                        pt = psS.tile([128, EOUT], F32, tag="sc", bufs=3, name="pt")
                        st = oTt[:, t * 128 : (t + 1) * 128]
                        nc.tensor.matmul(
                            pt[:, :512], st, wp[:, :512], start=True, stop=True
                        )
                        nc.tensor.matmul(
                            pt[:, 512:EOUT],
                            st,
                            wp[:, 512:EOUT],
                            start=True,
                            stop=True,
                        )
                        if t == 0:
                            ysb_tiles[b] = yp.tile(
                                [128, 4, EOUT], BF16, tag="ysb", bufs=2, name="ysb"
                            )
                        nc.vector.tensor_copy(ysb_tiles[b][:, t, :], pt[:])
                        if b == nqb - 1:
                            # last block: drain piecewise so the final DMA
                            # only waits on the final cast
                            nc.sync.dma_start(
                                y_d[b, :, t, :], ysb_tiles[b][:, t, :]
                            )
                        elif t == 3:
                            # whole block rides out in one contiguous DMA
                            # (dram is [block][partition][piece][col]; host
                            # untangles the piece/partition order)
                            nc.sync.dma_start(y_d[b], ysb_tiles[b][:])
                        spent += 1
                    avq.pop(0)

            def push_block_done(b):
                avq.append(("cast", b))
                for t in range(4):
                    avq.append(("proj", b, t))

            def emit_pair(b, j):
                qs = slice(b * 512, (b + 1) * 512)
                sc = psS.tile([128, 2, 512], F32, tag="sc", bufs=3, name="sc")
                for i in range(2):
                    kc = 2 * j + i
                    nc.tensor.matmul(
                        sc[:, i, :],
                        kT[:, kc * 128 : (kc + 1) * 128],
                        qT[:, qs],
                        start=True,
                        stop=True,
                    )
                E = ep.tile([128, 2, 512], FP8, tag="E", bufs=24, name="E")
                if b == nqb - 1 and j >= npair - 2:
                    # tail: halve the last exps across both engines so the
                    # final AV/cast/proj chain starts sooner
                    nc.scalar.activation(E[:, 0, :], sc[:, 0, :], AF.Exp)
                    nc.vector.tensor_scalar(
                        E[:, 1, :].bitcast(I8), sc[:, 1, :], A_SCH, B_SCH,
                        MULT, ADD,
                    )
                elif j in (DVE_PAIRS_B0 if b == 0 else DVE_PAIRS):
                    nc.vector.tensor_scalar(
                        E[:].bitcast(I8), sc[:], A_SCH, B_SCH, MULT, ADD
                    )
                else:
                    nc.scalar.activation(E[:], sc[:], AF.Exp)
                avq.append(("av", b, E, j))

            # ---------------- Phase A + block-0 scores ----------------
            with tc.tile_pool(name="psA", bufs=1, space="PSUM") as psA:
                # PE p-state warmup through the sc ring while x DMA lands
                wps = psS.tile([128, 2, 512], F32, tag="sc", bufs=3, name="sc")
                for _ in range(90):
                    nc.tensor.matmul(
                        wps[0:16, 0, 0:16], warm[:], warm[:], start=True, stop=True
                    )

                def v_iter(t0):
                    psvs = [
                        psA.tile([128, HD + 2], F32, tag="qk", bufs=2, name="ps_v")
                        for _ in range(2)
                    ]
                    for c in range(NCH):
                        for i in range(2):
                            ts_ = slice((t0 + i) * 128, (t0 + i + 1) * 128)
                            nc.tensor.matmul(
                                psvs[i][:],
                                xT[:, c, ts_],
                                wv[:, c, :],
                                start=(c == 0),
                                stop=(c == NCH - 1),
                            )
                    for i in range(2):
                        nc.vector.tensor_copy(
                            vaug[:, t0 + i, 0 : HD + 2], psvs[i][:]
                        )

                # two v iterations over block-0 tokens fill the gap while
                # the rest of x streams in
                v_iter(0)
                v_iter(2)

                for b in range(nqb):
                    qs = slice(b * 512, (b + 1) * 512)
                    ps_q = psA.tile([128, 512], F32, tag="qk", bufs=2, name="ps_q")
                    ps_k = psA.tile([128, 512], F32, tag="qk", bufs=2, name="ps_k")
                    for c in range(NCH):
                        for w, ps in ((wq, ps_q), (wk, ps_k)):
                            nc.tensor.matmul(
                                ps[:],
                                w[:, c, :],
                                xT[:, c, qs],
                                start=(c == 0),
                                stop=(c == NCH - 1),
                            )
                    nc.scalar.activation(qT[:, qs], ps_q[:], AF.Copy)
                    nc.vector.tensor_copy(kT[:, qs], ps_k[:])
                    # block 0's score pairs over the k-chunks this qk block
                    # just produced
                    emit_pair(0, 2 * b)
                    emit_pair(0, 2 * b + 1)
                # block 0's deferred work queues ahead of block 1's early
                # AVs so cast(0) releases the psO ring before they pop
                push_block_done(0)
                # v: two token-block chains in flight; six of block 1's
                # score pairs interleave so ScalarE/DVE keep working
                # through the v-pass
                early_b1 = {4: 0, 8: 1, 12: 2, 16: 3, 20: 4, 24: 5, 28: 6}
                for t0 in range(4, nkc, 2):
                    v_iter(t0)
                    if t0 in early_b1:
                        emit_pair(1, early_b1[t0])

            # ------------- Phase B: blocks 1-7 + deferred work -------------
            with tc.tile_pool(name="psO", bufs=1, space="PSUM") as psO:
                for b in range(1, nqb):
                    last = b == nqb - 1
                    for j in range(npair):
                        if b == 1 and j < 7:
                            continue  # emitted during the v-pass
                        emit_pair(b, j)
                        pop_work(3 if last else 2, floor=0 if last else 2)
                    push_block_done(b)
                # drain
                pop_work(10**9)

    nc.compile()
    return nc


def _prep_inputs(x, w_qkv, b_qkv, w_proj, nt):
    """Host-side shard prep: returns list of 8 in_maps."""
    x = np.asarray(x, dtype=np.float32)
    w_qkv = np.asarray(w_qkv, dtype=np.float32)
    b_qkv = np.asarray(b_qkv, dtype=np.float32)
    w_proj = np.asarray(w_proj, dtype=np.float32)

    xt = x.reshape(nt, EMBED)
    xT_pad = np.zeros((NCH * 128, nt), dtype=np.float32)
    xT_pad[:EMBED] = xt.T
    xT_pad[EMBED] = 1.0
    # [128, NCH, nt]: partition-major to match the SBUF tile layout
    xT_in = np.ascontiguousarray(
        xT_pad.reshape(NCH, 128, nt).transpose(1, 0, 2)
    ).astype(BF16_NP)

    s = float(HD) ** -0.5
    in_maps = []
    for h in range(NHEADS):
        sl_q = slice(h * HD, (h + 1) * HD)
        sl_k = slice(EMBED + h * HD, EMBED + (h + 1) * HD)
        sl_v = slice(2 * EMBED + h * HD, 2 * EMBED + (h + 1) * HD)

        wq_t = np.zeros((NCH * 128, 128), dtype=np.float32)
        wq_t[:EMBED, :HD] = (w_qkv[sl_q] * s).T
        wq_t[EMBED, :HD] = b_qkv[sl_q] * s

        wk_t = np.zeros((NCH * 128, 128), dtype=np.float32)
        wk_t[:EMBED, :HD] = w_qkv[sl_k].T
        wk_t[EMBED, :HD] = b_qkv[sl_k]

        # ones column at index 0 so the softmax denominator lands on
        # PSUM partition 0 (-> oT row 0)
        wv_t = np.zeros((NCH * 128, HD + 2), dtype=np.float32)
        wv_t[:EMBED, 1 : HD + 1] = w_qkv[sl_v].T
        wv_t[EMBED, 1 : HD + 1] = b_qkv[sl_v]
        wv_t[EMBED, 0] = 1.0

        # proj weights: row 0 = denom row: zero into data cols, 1.0 into
        # col 528 so y[:, 528] = softmax denominator per token
        wp_t = np.zeros((128, EOUT), dtype=np.float32)
        wp_t[1 : HD + 1, :EMBED] = w_proj[:, sl_q].T
        wp_t[0, EMBED] = 1.0

        in_maps.append(
            {
                "xT": xT_in,
                "wq": np.ascontiguousarray(
                    wq_t.reshape(NCH, 128, 128).transpose(1, 0, 2)
                ).astype(BF16_NP),
                "wk": np.ascontiguousarray(
                    wk_t.reshape(NCH, 128, 128).transpose(1, 0, 2)
                ).astype(BF16_NP),
                "wv": np.ascontiguousarray(
                    wv_t.reshape(NCH, 128, HD + 2).transpose(1, 0, 2)
                ).astype(BF16_NP),
                "wp": wp_t.astype(BF16_NP),
            }
        )
    return in_maps


_NC_CACHE = {}


def _get_nc(nt=NT):
    if nt not in _NC_CACHE:
        _NC_CACHE[nt] = _build_nc(nt)
    return _NC_CACHE[nt]


def kernel(x, w_qkv, b_qkv, w_proj, b_proj, _trace=False):
    from concourse.bass_utils import run_bass_kernel_spmd

    x = np.asarray(x, dtype=np.float32)
    b_proj = np.asarray(b_proj, dtype=np.float32)
    B, D, H, W, C = x.shape
    nt = D * H * W

    nc = _get_nc(nt)
    in_maps = _prep_inputs(x, w_qkv, b_qkv, w_proj, nt)
    res = run_bass_kernel_spmd(
        nc, in_maps, core_ids=list(range(NHEADS)), trace=_trace
    )
    out = np.zeros((nt, EMBED), dtype=np.float32)
    for r in res.results:
        yraw = np.asarray(r["y"], dtype=np.float32)
        # [block][partition][piece][col] -> row = block*512 + piece*128 + p
        yfull = yraw.transpose(0, 2, 1, 3).reshape(nt, EOUT)
        out += yfull[:, :EMBED] / yfull[:, EMBED : EMBED + 1]
    out += b_proj
    kernel.last_results = res
    return out.reshape(B, D, H, W, C)


# revision 28
# speedup vs baseline: 1.0023x; 1.0023x over previous
"""Trainium2 Bass kernel for 3D multi-head attention (nn_Attention3D).

Problem: x [1, 16, 16, 16, 528] -> full attention over N=4096 tokens,
8 heads of dim 66, qkv + out projections.

Sharding: one head per NeuronCore (8 cores). Each core computes its
head's q/k/v projections, full 4096x4096 attention, and its partial
contribution to the output projection. Host divides each core's
partial by its softmax denominator (carried as an extra output
column), sums the 8 partials and adds the output bias.

Final pipeline (fp8 AV + dual-engine exp; ~171us HW, vs 201us v1):
  - scores and projections stay bf16: on this silicon fp8 DoubleRow
    streams one moving column per cycle (like bf16) and disables the
    fast-weight-load, so fp8 matmuls only pay off where they merge two
    instructions into one. AV does exactly that: each AV is one
    DoubleRow matmul over two 128-token k-chunks (vaug [128,2k,80]
    stationary, E [128,2,512] moving), halving AV instruction count.
  - exp of the 16.7M scores is split across two engines (the ~109us
    ScalarE-only exp was the v1 co-bottleneck): ScalarE runs native
    Exp into fp8 E tiles (~10/16 pairs per block), DVE runs a one-op
    Schraudolph on the rest: int8(A*s + B) written through a bitcast
    into the fp8 tile -- the int8 grid IS the fp8e4m3
    exponent/mantissa grid, so the linear-in-log approximation lands
    within fp8 rounding error. DVE also carries the y/oT/kT/v casts,
    ScalarE the qT casts.
  - scores are emitted in uniform 2-chunk pairs ([128,2,512] f32 PSUM
    = 2 banks, ring of 3, shared with the out-proj pieces); o_ps
    [80,512] ring-2; the deferred-work FIFO (AV, oT cast, out-proj
    pieces) replays a block behind the score/exp pipeline.
  - v is quantized to fp8 at the PSUM->SBUF copy into an 80-stride
    layout (DoubleRow k-tile step must be %16==0), with a ones column
    so the softmax denominator rides the AV accumulator row 0.
  - x loads in a few wide strided DMAs (per-dma_start issue costs
    ~650ns on SyncE); 90 warmup matmuls hold the PE p-state until
    block 0's x lands; y leaves as one contiguous DMA per 512-token
    block in a [block][partition][piece][col] dram layout the host
    untangles; the last block drains piecewise and splits its final
    exps across both engines to shorten the tail.
Phase B runs at the PE streaming roofline (~14.4us per 512-token
block: 16384 score cols + 16384 AV cols at ~2.4GHz, 1 col/cycle).
Measured rel err vs fp32 reference ~8.3e-3.
"""

import numpy as np

import ml_dtypes

BF16_NP = ml_dtypes.bfloat16
FP8_NP = ml_dtypes.float8_e4m3

EMBED = 528
EOUT = 536  # proj output cols: 528 data + denom col (528) + pad
HD = 66
NHEADS = 8
NT = 4096
NCH = 5  # contraction chunks of 128 (640 = 528 + bias row + pad)

# Schraudolph fast-exp constants: scores arrive pre-scaled by hd^-0.5
# (folded into wq), so A is just 8*log2(e) onto the int8/fp8e4m3 grid.
A_SCH = 8.0 * np.log2(np.e)
B_SCH = 56.0 - 0.35

# exp-engine assignment per score pair (16 pairs of k-chunks per block):
# pairs listed go to DVE (Schraudolph), the rest to ScalarE Exp. DVE
# also carries the y/oT/kT/v casts, so it gets the smaller share.
DVE_PAIRS_B0 = frozenset({1, 3, 5, 7, 9, 11, 13})  # 7/16 in block 0
DVE_PAIRS = frozenset({1, 4, 7, 10, 12, 15})  # 6/16 in blocks 1-7


def _build_nc(nt=NT):
    import concourse.tile as tile
    from concourse import bacc, mybir

    F32 = mybir.dt.float32
    BF16 = mybir.dt.bfloat16
    FP8 = mybir.dt.float8e4
    I8 = mybir.dt.int8
    AF = mybir.ActivationFunctionType
    DR = mybir.MatmulPerfMode.DoubleRow
    MULT = mybir.AluOpType.mult
    ADD = mybir.AluOpType.add

    nkc = nt // 128  # k-token chunks (32)
    npair = nkc // 2  # AV pairs per block (16)
    nqb = nt // 512  # q-token blocks (8)

    nc = bacc.Bacc("TRN2", target_bir_lowering=False, debug=False)
    xT_d = nc.dram_tensor("xT", [128, NCH, nt], BF16, kind="ExternalInput").ap()
    wq_d = nc.dram_tensor("wq", [128, NCH, 128], BF16, kind="ExternalInput").ap()
    wk_d = nc.dram_tensor("wk", [128, NCH, 128], BF16, kind="ExternalInput").ap()
    wv_d = nc.dram_tensor("wv", [128, NCH, HD + 2], BF16, kind="ExternalInput").ap()
    wp_d = nc.dram_tensor("wp", [128, EOUT], BF16, kind="ExternalInput").ap()
    y_d = nc.dram_tensor(
        "y", [nt // 512, 128, 4, EOUT], BF16, kind="ExternalOutput"
    ).ap()

    with tile.TileContext(nc) as tc:
        with (
            tc.tile_pool(name="const", bufs=1) as constp,
            tc.tile_pool(name="persist", bufs=1) as pp,
            tc.tile_pool(name="ep", bufs=8) as ep,
            tc.tile_pool(name="yp", bufs=4) as yp,
            tc.tile_pool(name="psS", bufs=1, space="PSUM") as psS,
        ):
            wq = constp.tile([128, NCH, 128], BF16, name="wq_sb")
            wk = constp.tile([128, NCH, 128], BF16, name="wk_sb")
            wv = constp.tile([128, NCH, HD + 2], BF16, name="wv_sb")
            wp = constp.tile([128, EOUT], BF16, name="wp_sb")
            warm = constp.tile([128, 16], BF16, name="warm_sb")

            xT = pp.tile([128, NCH, nt], BF16, name="xT_sb")
            # block 0's x + qk weights first so qk(0) starts ~9us in;
            # the rest of x in wide DMAs that land on other queues
            nc.sync.dma_start(wq[:], wq_d[:])
            nc.sync.dma_start(xT[:, 0:2, 0:512], xT_d[:, 0:2, 0:512])
            nc.sync.dma_start(xT[:, 2:NCH, 0:512], xT_d[:, 2:NCH, 0:512])
            nc.sync.dma_start(wk[:], wk_d[:])
            nc.sync.dma_start(wv[:], wv_d[:])
            nc.sync.dma_start(xT[:, :, 512:2048], xT_d[:, :, 512:2048])
            nc.sync.dma_start(xT[:, :, 2048:nt], xT_d[:, :, 2048:nt])
            nc.sync.dma_start(wp[:], wp_d[:])

            # qT/kT are hd-padded to 128 partitions (rows HD.. stay 0) so
            # scores contract over a full K=128.
            qT = pp.tile([128, nt], BF16, name="qT")
            kT = pp.tile([128, nt], BF16, name="kT")
            # v in fp8 with an 80-byte chunk stride (DoubleRow k-tile dim
            # step must be a multiple of 16); cols 68-79 stay zero.
            vaug = pp.tile([128, nkc, 80], FP8, name="vaug")
            # out-proj stationary per block, double-buffered; rows 68-127
            # must read zero in the proj matmul, so memset once and only
            # ever write rows 0..67.
            oT = [pp.tile([128, 512], BF16, name=f"oT{i}") for i in range(2)]
            nc.gpsimd.memset(warm[:], 0)
            nc.gpsimd.memset(vaug[:], 0)
            nc.gpsimd.memset(oT[0][:], 0)
            nc.gpsimd.memset(oT[1][:], 0)

            # ---- deferred-work FIFO: AV pairs, oT casts, projections ----
            o_ps_tiles = {}
            ysb_tiles = {}
            avq = []

            def pop_work(budget, floor=0):
                spent = 0
                while len(avq) > floor and spent < budget:
                    item = avq[0]
                    kind = item[0]
                    if kind == "av":
                        _, b, E, j = item
                        if b not in o_ps_tiles:
                            o_ps_tiles[b] = psO.tile(
                                [80, 512], F32, tag="o", bufs=2, name="o_ps"
                            )
                        o_ps = o_ps_tiles[b]
                        nc.tensor.matmul(
                            o_ps[:],
                            vaug[:, 2 * j : 2 * j + 2, :],
                            E[:],
                            start=(j == 0),
                            stop=(j == npair - 1),
                            perf_mode=DR,
                            skip_group_check=True,
                        )
                        spent += 1
                    elif kind == "cast":
                        b = item[1]
                        nc.vector.tensor_copy(
                            oT[b % 2][0 : HD + 2, :], o_ps_tiles[b][0 : HD + 2, :]
                        )
                    else:  # proj piece: one 128-token slice of block b
                        _, b, t = item
                        oTt = oT[b % 2]
                        pt = psO.tile([128, EOUT], F32, tag="pt", bufs=1, name="pt")
                        st = oTt[:, t * 128 : (t + 1) * 128]
                        nc.tensor.matmul(
                            pt[:, :512], st, wp[:, :512], start=True, stop=True
                        )
                        nc.tensor.matmul(
                            pt[:, 512:EOUT],
                            st,
                            wp[:, 512:EOUT],
                            start=True,
                            stop=True,
                        )
                        if t == 0:
                            ysb_tiles[b] = yp.tile(
                                [128, 4, EOUT], BF16, tag="ysb", bufs=2, name="ysb"
                            )
                        if b == nqb - 1 and t % 2 == 0:
                            nc.scalar.activation(
                                ysb_tiles[b][:, t, :], pt[:], AF.Copy
                            )
                        else:
                            nc.vector.tensor_copy(ysb_tiles[b][:, t, :], pt[:])
                        if b == nqb - 1:
                            # last block: drain piecewise so the final DMA
                            # only waits on the final cast
                            nc.sync.dma_start(
                                y_d[b, :, t, :], ysb_tiles[b][:, t, :]
                            )
                        elif t == 3:
                            # whole block rides out in one contiguous DMA
                            # (dram is [block][partition][piece][col]; host
                            # untangles the piece/partition order)
                            nc.sync.dma_start(y_d[b], ysb_tiles[b][:])
                        spent += 1
                    avq.pop(0)

            def push_block_done(b):
                avq.append(("cast", b))
                for t in range(4):
                    avq.append(("proj", b, t))

            def emit_pair(b, j):
                qs = slice(b * 512, (b + 1) * 512)
                sc = psS.tile([128, 2, 512], F32, tag="sc", bufs=2, name="sc")
                for i in range(2):
                    kc = 2 * j + i
                    nc.tensor.matmul(
                        sc[:, i, :],
                        kT[:, kc * 128 : (kc + 1) * 128],
                        qT[:, qs],
                        start=True,
                        stop=True,
                    )
                E = ep.tile([128, 2, 512], FP8, tag="E", bufs=24, name="E")
                if b == nqb - 1 and j >= npair - 2:
                    # tail: halve the last exps across both engines so the
                    # final AV/cast/proj chain starts sooner
                    nc.scalar.activation(E[:, 0, :], sc[:, 0, :], AF.Exp)
                    nc.vector.tensor_scalar(
                        E[:, 1, :].bitcast(I8), sc[:, 1, :], A_SCH, B_SCH,
                        MULT, ADD,
                    )
                elif j in (DVE_PAIRS_B0 if b == 0 else DVE_PAIRS):
                    nc.vector.tensor_scalar(
                        E[:].bitcast(I8), sc[:], A_SCH, B_SCH, MULT, ADD
                    )
                else:
                    nc.scalar.activation(E[:], sc[:], AF.Exp)
                avq.append(("av", b, E, j))

            # ---------------- Phase A + block-0 scores ----------------
            with tc.tile_pool(name="psA", bufs=1, space="PSUM") as psA:
                # PE p-state warmup through the sc ring while x DMA lands
                wps = psS.tile([128, 2, 512], F32, tag="sc", bufs=2, name="sc")
                for _ in range(90):
                    nc.tensor.matmul(
                        wps[0:16, 0, 0:16], warm[:], warm[:], start=True, stop=True
                    )

                def v_iter(t0):
                    psvs = [
                        psA.tile([128, HD + 2], F32, tag="qk", bufs=4, name="ps_v")
                        for _ in range(2)
                    ]
                    for c in range(NCH):
                        for i in range(2):
                            ts_ = slice((t0 + i) * 128, (t0 + i + 1) * 128)
                            nc.tensor.matmul(
                                psvs[i][:],
                                xT[:, c, ts_],
                                wv[:, c, :],
                                start=(c == 0),
                                stop=(c == NCH - 1),
                            )
                    for i in range(2):
                        nc.vector.tensor_copy(
                            vaug[:, t0 + i, 0 : HD + 2], psvs[i][:]
                        )

                # two v iterations over block-0 tokens fill the gap while
                # the rest of x streams in
                v_iter(0)
                v_iter(2)

                for b in range(nqb):
                    qs = slice(b * 512, (b + 1) * 512)
                    ps_q = psA.tile([128, 512], F32, tag="qk", bufs=4, name="ps_q")
                    ps_k = psA.tile([128, 512], F32, tag="qk", bufs=4, name="ps_k")
                    for c in range(NCH):
                        for w, ps in ((wq, ps_q), (wk, ps_k)):
                            nc.tensor.matmul(
                                ps[:],
                                w[:, c, :],
                                xT[:, c, qs],
                                start=(c == 0),
                                stop=(c == NCH - 1),
                            )
                    nc.scalar.activation(qT[:, qs], ps_q[:], AF.Copy)
                    nc.vector.tensor_copy(kT[:, qs], ps_k[:])
                    # block 0's score pairs over the k-chunks this qk block
                    # just produced
                    emit_pair(0, 2 * b)
                    emit_pair(0, 2 * b + 1)
                # block 0's deferred work queues ahead of block 1's early
                # AVs so cast(0) releases the psO ring before they pop
                push_block_done(0)
                # v: two token-block chains in flight; six of block 1's
                # score pairs interleave so ScalarE/DVE keep working
                # through the v-pass
                early_b1 = {4: 0, 8: 1, 12: 2, 16: 3, 20: 4, 24: 5, 28: 6}
                for t0 in range(4, nkc, 2):
                    v_iter(t0)
                    if t0 in early_b1:
                        emit_pair(1, early_b1[t0])

            # ------------- Phase B: blocks 1-7 + deferred work -------------
            with tc.tile_pool(name="psO", bufs=1, space="PSUM") as psO:
                for b in range(1, nqb):
                    last = b == nqb - 1
                    for j in range(npair):
                        if b == 1 and j < 7:
                            continue  # emitted during the v-pass
                        emit_pair(b, j)
                        pop_work(3 if last else 2, floor=0 if last else 2)
                    push_block_done(b)
                # drain
                pop_work(10**9)

    nc.compile()
    return nc


def _prep_inputs(x, w_qkv, b_qkv, w_proj, nt):
    """Host-side shard prep: returns list of 8 in_maps."""
    x = np.asarray(x, dtype=np.float32)
    w_qkv = np.asarray(w_qkv, dtype=np.float32)
    b_qkv = np.asarray(b_qkv, dtype=np.float32)
    w_proj = np.asarray(w_proj, dtype=np.float32)

    xt = x.reshape(nt, EMBED)
    xT_pad = np.zeros((NCH * 128, nt), dtype=np.float32)
    xT_pad[:EMBED] = xt.T
    xT_pad[EMBED] = 1.0
    # [128, NCH, nt]: partition-major to match the SBUF tile layout
    xT_in = np.ascontiguousarray(
        xT_pad.reshape(NCH, 128, nt).transpose(1, 0, 2)
    ).astype(BF16_NP)

    s = float(HD) ** -0.5
    in_maps = []
    for h in range(NHEADS):
        sl_q = slice(h * HD, (h + 1) * HD)
        sl_k = slice(EMBED + h * HD, EMBED + (h + 1) * HD)
        sl_v = slice(2 * EMBED + h * HD, 2 * EMBED + (h + 1) * HD)

        wq_t = np.zeros((NCH * 128, 128), dtype=np.float32)
        wq_t[:EMBED, :HD] = (w_qkv[sl_q] * s).T
        wq_t[EMBED, :HD] = b_qkv[sl_q] * s

        wk_t = np.zeros((NCH * 128, 128), dtype=np.float32)
        wk_t[:EMBED, :HD] = w_qkv[sl_k].T
        wk_t[EMBED, :HD] = b_qkv[sl_k]

        # ones column at index 0 so the softmax denominator lands on
        # PSUM partition 0 (-> oT row 0)
        wv_t = np.zeros((NCH * 128, HD + 2), dtype=np.float32)
        wv_t[:EMBED, 1 : HD + 1] = w_qkv[sl_v].T
        wv_t[EMBED, 1 : HD + 1] = b_qkv[sl_v]
        wv_t[EMBED, 0] = 1.0

        # proj weights: row 0 = denom row: zero into data cols, 1.0 into
        # col 528 so y[:, 528] = softmax denominator per token
        wp_t = np.zeros((128, EOUT), dtype=np.float32)
        wp_t[1 : HD + 1, :EMBED] = w_proj[:, sl_q].T
        wp_t[0, EMBED] = 1.0

        in_maps.append(
            {
                "xT": xT_in,
                "wq": np.ascontiguousarray(
                    wq_t.reshape(NCH, 128, 128).transpose(1, 0, 2)
                ).astype(BF16_NP),
                "wk": np.ascontiguousarray(
                    wk_t.reshape(NCH, 128, 128).transpose(1, 0, 2)
                ).astype(BF16_NP),
                "wv": np.ascontiguousarray(
                    wv_t.reshape(NCH, 128, HD + 2).transpose(1, 0, 2)
                ).astype(BF16_NP),
                "wp": wp_t.astype(BF16_NP),
            }
        )
    return in_maps


_NC_CACHE = {}


def _get_nc(nt=NT):
    if nt not in _NC_CACHE:
        _NC_CACHE[nt] = _build_nc(nt)
    return _NC_CACHE[nt]


def kernel(x, w_qkv, b_qkv, w_proj, b_proj, _trace=False):
    from concourse.bass_utils import run_bass_kernel_spmd

    x = np.asarray(x, dtype=np.float32)
    b_proj = np.asarray(b_proj, dtype=np.float32)
    B, D, H, W, C = x.shape
    nt = D * H * W

    nc = _get_nc(nt)
    in_maps = _prep_inputs(x, w_qkv, b_qkv, w_proj, nt)
    res = run_bass_kernel_spmd(
        nc, in_maps, core_ids=list(range(NHEADS)), trace=_trace
    )
    out = np.zeros((nt, EMBED), dtype=np.float32)
    for r in res.results:
        yraw = np.asarray(r["y"], dtype=np.float32)
        # [block][partition][piece][col] -> row = block*512 + piece*128 + p
        yfull = yraw.transpose(0, 2, 1, 3).reshape(nt, EOUT)
        out += yfull[:, :EMBED] / yfull[:, EMBED : EMBED + 1]
    out += b_proj
    kernel.last_results = res
    return out.reshape(B, D, H, W, C)


# revision 29
# speedup vs baseline: 1.0045x; 1.0023x over previous
"""Trainium2 Bass kernel for 3D multi-head attention (nn_Attention3D).

Problem: x [1, 16, 16, 16, 528] -> full attention over N=4096 tokens,
8 heads of dim 66, qkv + out projections.

Sharding: one head per NeuronCore (8 cores). Each core computes its
head's q/k/v projections, full 4096x4096 attention, and its partial
contribution to the output projection. Host divides each core's
partial by its softmax denominator (carried as an extra output
column), sums the 8 partials and adds the output bias.

Final pipeline (fp8 AV + dual-engine exp; ~171us HW, vs 201us v1):
  - scores and projections stay bf16: on this silicon fp8 DoubleRow
    streams one moving column per cycle (like bf16) and disables the
    fast-weight-load, so fp8 matmuls only pay off where they merge two
    instructions into one. AV does exactly that: each AV is one
    DoubleRow matmul over two 128-token k-chunks (vaug [128,2k,80]
    stationary, E [128,2,512] moving), halving AV instruction count.
  - exp of the 16.7M scores is split across two engines (the ~109us
    ScalarE-only exp was the v1 co-bottleneck): ScalarE runs native
    Exp into fp8 E tiles (~10/16 pairs per block), DVE runs a one-op
    Schraudolph on the rest: int8(A*s + B) written through a bitcast
    into the fp8 tile -- the int8 grid IS the fp8e4m3
    exponent/mantissa grid, so the linear-in-log approximation lands
    within fp8 rounding error. DVE also carries the y/oT/kT/v casts,
    ScalarE the qT casts.
  - scores are emitted in uniform 2-chunk pairs ([128,2,512] f32 PSUM
    = 2 banks, ring of 3, shared with the out-proj pieces); o_ps
    [80,512] ring-2; the deferred-work FIFO (AV, oT cast, out-proj
    pieces) replays a block behind the score/exp pipeline.
  - v is quantized to fp8 at the PSUM->SBUF copy into an 80-stride
    layout (DoubleRow k-tile step must be %16==0), with a ones column
    so the softmax denominator rides the AV accumulator row 0.
  - x loads in a few wide strided DMAs (per-dma_start issue costs
    ~650ns on SyncE); 90 warmup matmuls hold the PE p-state until
    block 0's x lands; y leaves as one contiguous DMA per 512-token
    block in a [block][partition][piece][col] dram layout the host
    untangles; the last block drains piecewise and splits its final
    exps across both engines to shorten the tail.
Phase B runs at the PE streaming roofline (~14.4us per 512-token
block: 16384 score cols + 16384 AV cols at ~2.4GHz, 1 col/cycle).
Measured rel err vs fp32 reference ~8.3e-3.
"""

import numpy as np

import ml_dtypes

BF16_NP = ml_dtypes.bfloat16
FP8_NP = ml_dtypes.float8_e4m3

EMBED = 528
EOUT = 536  # proj output cols: 528 data + denom col (528) + pad
HD = 66
NHEADS = 8
NT = 4096
NCH = 5  # contraction chunks of 128 (640 = 528 + bias row + pad)

# Schraudolph fast-exp constants: scores arrive pre-scaled by hd^-0.5
# (folded into wq), so A is just 8*log2(e) onto the int8/fp8e4m3 grid.
A_SCH = 8.0 * np.log2(np.e)
B_SCH = 56.0 - 0.35

# exp-engine assignment per score pair (16 pairs of k-chunks per block):
# pairs listed go to DVE (Schraudolph), the rest to ScalarE Exp. DVE
# also carries the y/oT/kT/v casts, so it gets the smaller share.
DVE_PAIRS_B0 = frozenset({1, 3, 5, 7, 9, 11, 13})  # 7/16 in block 0
DVE_PAIRS = frozenset({1, 4, 7, 10, 12, 15})  # 6/16 in blocks 1-7


def _build_nc(nt=NT):
    import concourse.tile as tile
    from concourse import bacc, mybir

    F32 = mybir.dt.float32
    BF16 = mybir.dt.bfloat16
    FP8 = mybir.dt.float8e4
    I8 = mybir.dt.int8
    AF = mybir.ActivationFunctionType
    DR = mybir.MatmulPerfMode.DoubleRow
    MULT = mybir.AluOpType.mult
    ADD = mybir.AluOpType.add

    nkc = nt // 128  # k-token chunks (32)
    npair = nkc // 2  # AV pairs per block (16)
    nqb = nt // 512  # q-token blocks (8)

    nc = bacc.Bacc("TRN2", target_bir_lowering=False, debug=False)
    xT_d = nc.dram_tensor("xT", [128, NCH, nt], BF16, kind="ExternalInput").ap()
    wq_d = nc.dram_tensor("wq", [128, NCH, 128], BF16, kind="ExternalInput").ap()
    wk_d = nc.dram_tensor("wk", [128, NCH, 128], BF16, kind="ExternalInput").ap()
    wv_d = nc.dram_tensor("wv", [128, NCH, HD + 2], BF16, kind="ExternalInput").ap()
    wp_d = nc.dram_tensor("wp", [128, EOUT], BF16, kind="ExternalInput").ap()
    y_d = nc.dram_tensor(
        "y", [nt // 512, 128, 4, EOUT], BF16, kind="ExternalOutput"
    ).ap()

    with tile.TileContext(nc) as tc:
        with (
            tc.tile_pool(name="const", bufs=1) as constp,
            tc.tile_pool(name="persist", bufs=1) as pp,
            tc.tile_pool(name="ep", bufs=8) as ep,
            tc.tile_pool(name="yp", bufs=4) as yp,
            tc.tile_pool(name="psS", bufs=1, space="PSUM") as psS,
        ):
            wq = constp.tile([128, NCH, 128], BF16, name="wq_sb")
            wk = constp.tile([128, NCH, 128], BF16, name="wk_sb")
            wv = constp.tile([128, NCH, HD + 2], BF16, name="wv_sb")
            wp = constp.tile([128, EOUT], BF16, name="wp_sb")
            warm = constp.tile([128, 16], BF16, name="warm_sb")

            xT = pp.tile([128, NCH, nt], BF16, name="xT_sb")
            # block 0's x + qk weights first so qk(0) starts ~9us in;
            # the rest of x in wide DMAs that land on other queues
            nc.sync.dma_start(wv[:], wv_d[:])
            nc.sync.dma_start(xT[:, 0:2, 0:512], xT_d[:, 0:2, 0:512])
            nc.sync.dma_start(wq[:], wq_d[:])
            nc.sync.dma_start(xT[:, 2:NCH, 0:512], xT_d[:, 2:NCH, 0:512])
            nc.sync.dma_start(wk[:], wk_d[:])
            nc.sync.dma_start(xT[:, :, 512:2048], xT_d[:, :, 512:2048])
            nc.sync.dma_start(xT[:, :, 2048:nt], xT_d[:, :, 2048:nt])
            nc.sync.dma_start(wp[:], wp_d[:])

            # qT/kT are hd-padded to 128 partitions (rows HD.. stay 0) so
            # scores contract over a full K=128.
            qT = pp.tile([128, nt], BF16, name="qT")
            kT = pp.tile([128, nt], BF16, name="kT")
            # v in fp8 with an 80-byte chunk stride (DoubleRow k-tile dim
            # step must be a multiple of 16); cols 68-79 stay zero.
            vaug = pp.tile([128, nkc, 80], FP8, name="vaug")
            # out-proj stationary per block, double-buffered; rows 68-127
            # must read zero in the proj matmul, so memset once and only
            # ever write rows 0..67.
            oT = [pp.tile([128, 512], BF16, name=f"oT{i}") for i in range(2)]
            nc.gpsimd.memset(warm[:], 0)
            nc.gpsimd.memset(vaug[:], 0)
            nc.gpsimd.memset(oT[0][:], 0)
            nc.gpsimd.memset(oT[1][:], 0)

            # ---- deferred-work FIFO: AV pairs, oT casts, projections ----
            o_ps_tiles = {}
            ysb_tiles = {}
            avq = []

            def pop_work(budget, floor=0):
                spent = 0
                while len(avq) > floor and spent < budget:
                    item = avq[0]
                    kind = item[0]
                    if kind == "av":
                        _, b, E, j = item
                        if b not in o_ps_tiles:
                            o_ps_tiles[b] = psO.tile(
                                [80, 512], F32, tag="o", bufs=2, name="o_ps"
                            )
                        o_ps = o_ps_tiles[b]
                        nc.tensor.matmul(
                            o_ps[:],
                            vaug[:, 2 * j : 2 * j + 2, :],
                            E[:],
                            start=(j == 0),
                            stop=(j == npair - 1),
                            perf_mode=DR,
                            skip_group_check=True,
                        )
                        spent += 1
                    elif kind == "cast":
                        b = item[1]
                        nc.vector.tensor_copy(
                            oT[b % 2][0 : HD + 2, :], o_ps_tiles[b][0 : HD + 2, :]
                        )
                    else:  # proj piece: one 128-token slice of block b
                        _, b, t = item
                        oTt = oT[b % 2]
                        pt = psO.tile([128, EOUT], F32, tag="pt", bufs=1, name="pt")
                        st = oTt[:, t * 128 : (t + 1) * 128]
                        nc.tensor.matmul(
                            pt[:, :512], st, wp[:, :512], start=True, stop=True
                        )
                        nc.tensor.matmul(
                            pt[:, 512:EOUT],
                            st,
                            wp[:, 512:EOUT],
                            start=True,
                            stop=True,
                        )
                        if t == 0:
                            ysb_tiles[b] = yp.tile(
                                [128, 4, EOUT], BF16, tag="ysb", bufs=2, name="ysb"
                            )
                        if b == nqb - 1 and t % 2 == 0:
                            nc.scalar.activation(
                                ysb_tiles[b][:, t, :], pt[:], AF.Copy
                            )
                        else:
                            nc.vector.tensor_copy(ysb_tiles[b][:, t, :], pt[:])
                        if b == nqb - 1:
                            # last block: drain piecewise so the final DMA
                            # only waits on the final cast
                            nc.sync.dma_start(
                                y_d[b, :, t, :], ysb_tiles[b][:, t, :]
                            )
                        elif t == 3:
                            # whole block rides out in one contiguous DMA
                            # (dram is [block][partition][piece][col]; host
                            # untangles the piece/partition order)
                            nc.sync.dma_start(y_d[b], ysb_tiles[b][:])
                        spent += 1
                    avq.pop(0)

            def push_block_done(b):
                avq.append(("cast", b))
                for t in range(4):
                    avq.append(("proj", b, t))

            def emit_pair(b, j):
                qs = slice(b * 512, (b + 1) * 512)
                sc = psS.tile([128, 2, 512], F32, tag="sc", bufs=2, name="sc")
                for i in range(2):
                    kc = 2 * j + i
                    nc.tensor.matmul(
                        sc[:, i, :],
                        kT[:, kc * 128 : (kc + 1) * 128],
                        qT[:, qs],
                        start=True,
                        stop=True,
                    )
                E = ep.tile([128, 2, 512], FP8, tag="E", bufs=24, name="E")
                if b == nqb - 1 and j >= npair - 2:
                    # tail: halve the last exps across both engines so the
                    # final AV/cast/proj chain starts sooner
                    nc.scalar.activation(E[:, 0, :], sc[:, 0, :], AF.Exp)
                    nc.vector.tensor_scalar(
                        E[:, 1, :].bitcast(I8), sc[:, 1, :], A_SCH, B_SCH,
                        MULT, ADD,
                    )
                elif j in (DVE_PAIRS_B0 if b == 0 else DVE_PAIRS):
                    nc.vector.tensor_scalar(
                        E[:].bitcast(I8), sc[:], A_SCH, B_SCH, MULT, ADD
                    )
                else:
                    nc.scalar.activation(E[:], sc[:], AF.Exp)
                avq.append(("av", b, E, j))

            # ---------------- Phase A + block-0 scores ----------------
            with tc.tile_pool(name="psA", bufs=1, space="PSUM") as psA:
                # PE p-state warmup through the sc ring while x DMA lands
                wps = psS.tile([128, 2, 512], F32, tag="sc", bufs=2, name="sc")
                for _ in range(90):
                    nc.tensor.matmul(
                        wps[0:16, 0, 0:16], warm[:], warm[:], start=True, stop=True
                    )

                def v_iter(t0):
                    psvs = [
                        psA.tile([128, HD + 2], F32, tag="qk", bufs=4, name="ps_v")
                        for _ in range(2)
                    ]
                    for c in range(NCH):
                        for i in range(2):
                            ts_ = slice((t0 + i) * 128, (t0 + i + 1) * 128)
                            nc.tensor.matmul(
                                psvs[i][:],
                                xT[:, c, ts_],
                                wv[:, c, :],
                                start=(c == 0),
                                stop=(c == NCH - 1),
                            )
                    for i in range(2):
                        nc.vector.tensor_copy(
                            vaug[:, t0 + i, 0 : HD + 2], psvs[i][:]
                        )

                # two v iterations over block-0 tokens fill the gap while
                # the rest of x streams in
                v_iter(0)
                v_iter(2)

                for b in range(nqb):
                    qs = slice(b * 512, (b + 1) * 512)
                    ps_q = psA.tile([128, 512], F32, tag="qk", bufs=4, name="ps_q")
                    ps_k = psA.tile([128, 512], F32, tag="qk", bufs=4, name="ps_k")
                    for c in range(NCH):
                        for w, ps in ((wq, ps_q), (wk, ps_k)):
                            nc.tensor.matmul(
                                ps[:],
                                w[:, c, :],
                                xT[:, c, qs],
                                start=(c == 0),
                                stop=(c == NCH - 1),
                            )
                    nc.scalar.activation(qT[:, qs], ps_q[:], AF.Copy)
                    nc.vector.tensor_copy(kT[:, qs], ps_k[:])
                    # block 0's score pairs over the k-chunks this qk block
                    # just produced
                    emit_pair(0, 2 * b)
                    emit_pair(0, 2 * b + 1)
                # block 0's deferred work queues ahead of block 1's early
                # AVs so cast(0) releases the psO ring before they pop
                push_block_done(0)
                # v: two token-block chains in flight; six of block 1's
                # score pairs interleave so ScalarE/DVE keep working
                # through the v-pass
                early_b1 = {4: 0, 8: 1, 12: 2, 16: 3, 20: 4, 24: 5, 28: 6}
                for t0 in range(4, nkc, 2):
                    v_iter(t0)
                    if t0 in early_b1:
                        emit_pair(1, early_b1[t0])

            # ------------- Phase B: blocks 1-7 + deferred work -------------
            with tc.tile_pool(name="psO", bufs=1, space="PSUM") as psO:
                for b in range(1, nqb):
                    last = b == nqb - 1
                    for j in range(npair):
                        if b == 1 and j < 7:
                            continue  # emitted during the v-pass
                        emit_pair(b, j)
                        pop_work(3 if last else 2, floor=0 if last else 2)
                    push_block_done(b)
                # drain
                pop_work(10**9)

    nc.compile()
    return nc


def _prep_inputs(x, w_qkv, b_qkv, w_proj, nt):
    """Host-side shard prep: returns list of 8 in_maps."""
    x = np.asarray(x, dtype=np.float32)
    w_qkv = np.asarray(w_qkv, dtype=np.float32)
    b_qkv = np.asarray(b_qkv, dtype=np.float32)
    w_proj = np.asarray(w_proj, dtype=np.float32)

    xt = x.reshape(nt, EMBED)
    xT_pad = np.zeros((NCH * 128, nt), dtype=np.float32)
    xT_pad[:EMBED] = xt.T
    xT_pad[EMBED] = 1.0
    # [128, NCH, nt]: partition-major to match the SBUF tile layout
    xT_in = np.ascontiguousarray(
        xT_pad.reshape(NCH, 128, nt).transpose(1, 0, 2)
    ).astype(BF16_NP)

    s = float(HD) ** -0.5
    in_maps = []
    for h in range(NHEADS):
        sl_q = slice(h * HD, (h + 1) * HD)
        sl_k = slice(EMBED + h * HD, EMBED + (h + 1) * HD)
        sl_v = slice(2 * EMBED + h * HD, 2 * EMBED + (h + 1) * HD)

        wq_t = np.zeros((NCH * 128, 128), dtype=np.float32)
        wq_t[:EMBED, :HD] = (w_qkv[sl_q] * s).T
        wq_t[EMBED, :HD] = b_qkv[sl_q] * s

        wk_t = np.zeros((NCH * 128, 128), dtype=np.float32)
        wk_t[:EMBED, :HD] = w_qkv[sl_k].T
        wk_t[EMBED, :HD] = b_qkv[sl_k]

        # ones column at index 0 so the softmax denominator lands on
        # PSUM partition 0 (-> oT row 0)
        wv_t = np.zeros((NCH * 128, HD + 2), dtype=np.float32)
        wv_t[:EMBED, 1 : HD + 1] = w_qkv[sl_v].T
        wv_t[EMBED, 1 : HD + 1] = b_qkv[sl_v]
        wv_t[EMBED, 0] = 1.0

        # proj weights: row 0 = denom row: zero into data cols, 1.0 into
        # col 528 so y[:, 528] = softmax denominator per token
        wp_t = np.zeros((128, EOUT), dtype=np.float32)
        wp_t[1 : HD + 1, :EMBED] = w_proj[:, sl_q].T
        wp_t[0, EMBED] = 1.0

        in_maps.append(
            {
                "xT": xT_in,
                "wq": np.ascontiguousarray(
                    wq_t.reshape(NCH, 128, 128).transpose(1, 0, 2)
                ).astype(BF16_NP),
                "wk": np.ascontiguousarray(
                    wk_t.reshape(NCH, 128, 128).transpose(1, 0, 2)
                ).astype(BF16_NP),
                "wv": np.ascontiguousarray(
                    wv_t.reshape(NCH, 128, HD + 2).transpose(1, 0, 2)
                ).astype(BF16_NP),
                "wp": wp_t.astype(BF16_NP),
            }
        )
    return in_maps


_NC_CACHE = {}


def _get_nc(nt=NT):
    if nt not in _NC_CACHE:
        _NC_CACHE[nt] = _build_nc(nt)
    return _NC_CACHE[nt]


def kernel(x, w_qkv, b_qkv, w_proj, b_proj, _trace=False):
    from concourse.bass_utils import run_bass_kernel_spmd

    x = np.asarray(x, dtype=np.float32)
    b_proj = np.asarray(b_proj, dtype=np.float32)
    B, D, H, W, C = x.shape
    nt = D * H * W

    nc = _get_nc(nt)
    in_maps = _prep_inputs(x, w_qkv, b_qkv, w_proj, nt)
    res = run_bass_kernel_spmd(
        nc, in_maps, core_ids=list(range(NHEADS)), trace=_trace
    )
    out = np.zeros((nt, EMBED), dtype=np.float32)
    for r in res.results:
        yraw = np.asarray(r["y"], dtype=np.float32)
        # [block][partition][piece][col] -> row = block*512 + piece*128 + p
        yfull = yraw.transpose(0, 2, 1, 3).reshape(nt, EOUT)
        out += yfull[:, :EMBED] / yfull[:, EMBED : EMBED + 1]
    out += b_proj
    kernel.last_results = res
    return out.reshape(B, D, H, W, C)
